# revision 7
# baseline (speedup 1.0000x reference)
"""Trainium2 Bass kernel for nn_DictionaryLearning (batched OMP vector quantizer).

Self-contained: builds one SPMD NeuronCore program (data-parallel over the 8
batches of z_e, one batch per core), runs it on 8 cores via
run_bass_kernel_spmd, and assembles the full outputs on the host.

Per-core device algorithm (4096 patches = 32 tiles x 128):
  h_bar = patches @ D on the PE (z_e's [C, H*W] layout is already the lhsT).
  OMP iterations k = 0..4:
    argmax |h| per patch         -> DVE max / max_index
    gather u_k = D^T[j]          -> GPSIMD indirect DMA row gather
    Gram entries u_a.u_k, p.u_k  -> DVE segmented dots
    batched (k+1)x(k+1) solve    -> [128,32]-wide symmetric Gaussian elim (DVE)
    residual r = p - sum x_a u_a -> DVE
    h = r @ D                    -> PE (transpose r on PE first)
  Outputs: recon^T [64,4096] (= z_dl batch slice), atom indices, coefficients.

Host: loss = 1.25 * mean((recon - z_e)^2); coeffs scattered into [512, 32768].
"""
import sys
sys.path.insert(0, "/opt/trn_rl_repo")
import numpy as np
import concourse.bass as bass
import concourse.bacc as bacc_mod
import concourse.mybir as mybir
import concourse.tile as tile
from concourse.bass_utils import run_bass_kernel_spmd
from concourse.masks import make_identity

f32 = mybir.dt.float32
u32 = mybir.dt.uint32
Alu = mybir.AluOpType
Ax = mybir.AxisListType
Act = mybir.ActivationFunctionType

B = 8           # batches = cores
T = 32          # patch tiles per core
K = 5           # sparsity
C = 64          # atom dim
N = 512         # dictionary atoms
HW = T * 128    # patches per core
EPS = 1e-5      # Gram regularizer
COMMIT = 0.25   # commitment cost


def _emit_solve(nc, scal2, xbuf, tmp_pool, n):
    """Batched symmetric Gaussian elimination on [128, 32]-wide scalars.

    scal2[:, 5a+b, :] holds G_sub[a][b] for a<=b (upper, destroyed);
    scal2[:, 25+b, :] the rhs (destroyed); xbuf[:, a, :] gets the solution.
    """
    A = lambda a, b: scal2[:, 5 * a + b, :]
    Arow = lambda a, b0, b1: scal2[:, 5 * a + b0:5 * a + b1, :]
    rh = lambda a: scal2[:, 25 + a, :]
    recs = [tmp_pool.tile([128, 32], f32, tag="rec%d" % i, name="rec%d" % i)
            for i in range(n)]
    if n == 1:
        nc.vector.reciprocal(recs[0][:], A(0, 0))
        nc.vector.tensor_tensor(xbuf[:, 0, :], rh(0), recs[0][:], op=Alu.mult)
        return
    f = tmp_pool.tile([128, 1, 32], f32, tag="f")
    tmp = tmp_pool.tile([128, K, 32], f32, tag="tmp")
    for i in range(n):
        nc.vector.reciprocal(recs[i][:], A(i, i))
        for j in range(i + 1, n):
            nc.vector.tensor_tensor(f[:, 0, :], A(i, j), recs[i][:], op=Alu.mult)
            m = n - j
            nc.vector.tensor_tensor(tmp[:, :m, :], Arow(i, j, n),
                                    f[:].to_broadcast([128, m, 32]), op=Alu.mult)
            nc.vector.tensor_tensor(Arow(j, j, n), Arow(j, j, n), tmp[:, :m, :],
                                    op=Alu.subtract)
            nc.vector.tensor_tensor(tmp[:, 0, :], rh(i), f[:, 0, :], op=Alu.mult)
            nc.vector.tensor_tensor(rh(j), rh(j), tmp[:, 0, :], op=Alu.subtract)
    for i in range(n - 1, -1, -1):
        m = n - 1 - i
        if m > 0:
            nc.vector.tensor_tensor(tmp[:, :m, :], Arow(i, i + 1, n),
                                    xbuf[:, i + 1:n, :], op=Alu.mult)
            v = bass.AP(tmp[:].tensor, tmp[:].offset,
                        [tmp[:].ap[0], [1, 32], [32, m]])
            nc.vector.tensor_reduce(f[:, 0, :], v, axis=Ax.X, op=Alu.add)
            nc.vector.tensor_tensor(f[:, 0, :], rh(i), f[:, 0, :], op=Alu.subtract)
            nc.vector.tensor_tensor(xbuf[:, i, :], f[:, 0, :], recs[i][:], op=Alu.mult)
        else:
            nc.vector.tensor_tensor(xbuf[:, i, :], rh(i), recs[i][:], op=Alu.mult)


def build_core_program():
    nc = bacc_mod.Bacc(None)
    z_d = nc.dram_tensor("z", [C, HW], f32, kind="ExternalInput")
    d_d = nc.dram_tensor("d", [C, N], f32, kind="ExternalInput")
    dt_d = nc.dram_tensor("dt", [N, C], f32, kind="ExternalInput")
    rec_d = nc.dram_tensor("rec", [C, HW], f32, kind="ExternalOutput")
    jout_d = nc.dram_tensor("jout", [128, K, T], u32, kind="ExternalOutput")
    xout_d = nc.dram_tensor("xout", [128, K, T], f32, kind="ExternalOutput")

    with tile.TileContext(nc) as tc:
        with (
            tc.tile_pool(name="const", bufs=1) as cp,
            tc.tile_pool(name="sb", bufs=1) as sb,
            tc.tile_pool(name="rot", bufs=3) as rot,
            tc.tile_pool(name="hps", bufs=4, space="PSUM") as hps,
            tc.tile_pool(name="tps", bufs=2, space="PSUM") as tps,
        ):
            z_sb = cp.tile([C, HW], f32)
            d_sb = cp.tile([C, N], f32)
            ident = cp.tile([128, 128], f32)
            nc.gpsimd.dma_start(z_sb[:], z_d[:])
            nc.gpsimd.dma_start(d_sb[:], d_d[:])
            make_identity(nc, ident[:])

            ubuf = sb.tile([128, T, 6, C], f32)    # seg 0..4 = u_a, seg 5 = p_t
            jbuf = sb.tile([128, K, T, 8], u32)
            # rows 5a+b: G_sub[a][b] (upper used); rows 25+b: rhs p.u_b
            scal2 = sb.tile([128, 30, 32], f32)
            scw = sb.tile([128, 30, 32], f32)
            xbuf = sb.tile([128, K, 32], f32)
            rT_sb = sb.tile([C, T, 128], f32)
            nc.gpsimd.memset(ubuf[:], 0.0)

            for t in range(T):
                pt_ps = tps.tile([128, C], f32, tag="pt")
                nc.tensor.transpose(pt_ps[:], z_sb[:, t * 128:(t + 1) * 128],
                                    ident[0:C, 0:C])
                nc.scalar.copy(ubuf[:, t, 5, :], pt_ps[:])

            for k in range(K):
                for t in range(T):
                    h_ps = hps.tile([128, N], f32, tag="h")
                    lhsT = z_sb[:, t * 128:(t + 1) * 128] if k == 0 else rT_sb[:, t, :]
                    nc.tensor.matmul(h_ps[:], lhsT, d_sb[:], start=True, stop=True)
                    s = rot.tile([128, N], f32, tag="s")
                    nc.scalar.activation(s[:], h_ps[:], Act.Abs)
                    m8 = rot.tile([128, 8], f32, tag="m8")
                    nc.vector.max(m8[:], s[:])
                    nc.vector.max_index(jbuf[:, k, t, :], m8[:], s[:])
                    nc.gpsimd.indirect_dma_start(
                        out=ubuf[:, t, k, :], out_offset=None,
                        in_=dt_d[:],
                        in_offset=bass.IndirectOffsetOnAxis(
                            ap=jbuf[:, k, t, 0:1], axis=0))
                    # dots of u_k against [u_0..u_4, p]: one mul + one strided
                    # segmented reduce. Row (5a+k) gets G_sub[a][k] (a>k rows
                    # are lower-triangle junk, never read); row 25+k gets the
                    # rhs p.u_k.
                    prod = rot.tile([128, 6, C], f32, tag="prod")
                    nc.vector.tensor_tensor(
                        prod[:], ubuf[:, t, :, :],
                        ubuf[:, t, k:k + 1, :].to_broadcast([128, 6, C]),
                        op=Alu.mult)
                    out_ap = bass.AP(scal2[:].tensor,
                                     scal2[:].offset + (k * 32 + t),
                                     [scal2[:].ap[0], [5 * 32, 6]])
                    nc.vector.tensor_reduce(out_ap, prod[:],
                                            axis=Ax.X, op=Alu.add)
                # batched solve across all 32 tiles
                dpos = 5 * k + k
                nc.vector.tensor_scalar(scal2[:, dpos, :], scal2[:, dpos, :],
                                        EPS, None, op0=Alu.add)
                nc.vector.tensor_copy(scw[:], scal2[:])
                _emit_solve(nc, scw, xbuf, rot, k + 1)
                for t in range(T):
                    prod2 = rot.tile([128, K, C], f32, tag="prod2")
                    nc.vector.tensor_tensor(
                        prod2[:, 0:k + 1, :], ubuf[:, t, 0:k + 1, :],
                        xbuf[:, 0:k + 1, t:t + 1].to_broadcast([128, k + 1, C]),
                        op=Alu.mult)
                    ct = rot.tile([128, C], f32, tag="ct")
                    v = bass.AP(prod2[:].tensor, prod2[:].offset,
                                [prod2[:].ap[0], [1, C], [C, k + 1]])
                    nc.vector.tensor_reduce(ct[:], v, axis=Ax.X, op=Alu.add)
                    rt_ps = tps.tile([C, 128], f32, tag="rt")
                    if k < K - 1:
                        rt = rot.tile([128, C], f32, tag="rt_sb")
                        nc.vector.tensor_tensor(rt[:], ubuf[:, t, 5, :], ct[:],
                                                op=Alu.subtract)
                        nc.tensor.transpose(rt_ps[:], rt[:], ident[:])
                        nc.scalar.copy(rT_sb[:, t, :], rt_ps[:])
                    else:
                        nc.tensor.transpose(rt_ps[:], ct[:], ident[:])
                        crec = rot.tile([C, 128], f32, tag="crec")
                        nc.scalar.copy(crec[:], rt_ps[:])
                        nc.sync.dma_start(rec_d[:, t * 128:(t + 1) * 128], crec[:])

            nc.sync.dma_start(jout_d[:], jbuf[:, :, :, 0])
            nc.sync.dma_start(xout_d[:], xbuf[:])
    nc.finalize()
    return nc


_CACHED_NC = None


def kernel(z_e: np.ndarray, dictionary: np.ndarray):
    global _CACHED_NC
    z_e = np.ascontiguousarray(np.asarray(z_e, dtype=np.float32))
    D = np.ascontiguousarray(np.asarray(dictionary, dtype=np.float32))
    DT = np.ascontiguousarray(D.T)

    if _CACHED_NC is None:
        _CACHED_NC = build_core_program()
    nc = _CACHED_NC

    in_maps = []
    for b in range(B):
        zb = np.ascontiguousarray(z_e[b].reshape(C, HW))
        in_maps.append({"z": zb, "d": D, "dt": DT})
    res = run_bass_kernel_spmd(nc, in_maps, core_ids=list(range(B))).results

    z_dl = np.empty((B, C, 64, 64), dtype=np.float32)
    coeffsT = np.zeros((N, B * HW), dtype=np.float32)
    pidx = np.arange(HW)
    for b in range(B):
        rec = res[b]["rec"]                       # [C, HW]
        z_dl[b] = rec.reshape(C, 64, 64)
        jout = res[b]["jout"].astype(np.int64)    # [128, K, T]
        xout = res[b]["xout"]                     # [128, K, T]
        # patch p = t*128 + q  (q = partition)
        j_flat = jout.transpose(2, 0, 1).reshape(HW, K)   # [p, k]
        x_flat = xout.transpose(2, 0, 1).reshape(HW, K)
        for k in range(K):
            coeffsT[j_flat[:, k], b * HW + pidx] = x_flat[:, k]

    diff = z_dl.astype(np.float64) - z_e.astype(np.float64)
    loss = np.float32((1.0 + COMMIT) * np.mean(diff * diff))
    return z_dl, loss, coeffsT


# revision 20
# speedup vs baseline: 1.0864x; 1.0864x over previous
"""Trainium2 Bass kernel for nn_DictionaryLearning (batched OMP vector quantizer).

Self-contained: builds one SPMD NeuronCore program (data-parallel over the 8
batches of z_e, one batch per core), runs it on 8 cores via
run_bass_kernel_spmd, and assembles the full outputs on the host.

Per-core device algorithm (4096 patches = 32 tiles x 128):
  h_bar = patches @ D on the PE (z_e's [C, H*W] layout is already the lhsT).
  OMP iterations k = 0..4:
    argmax |h| per patch         -> DVE max / max_index
    gather u_k = D^T[j]          -> GPSIMD indirect DMA row gather
    Gram entries u_a.u_k, p.u_k  -> DVE segmented dots
    batched (k+1)x(k+1) solve    -> [128,32]-wide symmetric Gaussian elim (DVE)
    residual r = p - sum x_a u_a -> DVE
    h = r @ D                    -> PE (transpose r on PE first)
  Outputs: recon^T [64,4096] (= z_dl batch slice), atom indices, coefficients.

Host: loss = 1.25 * mean((recon - z_e)^2); coeffs scattered into [512, 32768].
"""
import sys
sys.path.insert(0, "/opt/trn_rl_repo")
import numpy as np
import concourse.bass as bass
import concourse.bacc as bacc_mod
import concourse.mybir as mybir
import concourse.tile as tile
from concourse.bass_utils import run_bass_kernel_spmd
from concourse.masks import make_identity

f32 = mybir.dt.float32
u32 = mybir.dt.uint32
Alu = mybir.AluOpType
Ax = mybir.AxisListType
Act = mybir.ActivationFunctionType

B = 8           # batches = cores
T = 32          # patch tiles per core
K = 5           # sparsity
C = 64          # atom dim
N = 512         # dictionary atoms
HW = T * 128    # patches per core
EPS = 1e-5      # Gram regularizer
COMMIT = 0.25   # commitment cost


def _emit_solve(nc, scal2, xbuf, tmp_pool, n):
    """Batched symmetric Gaussian elimination on [128, 32]-wide scalars.

    scal2[:, b, :] (b<5) holds the rhs p.u_b; scal2[:, 5+5a+b, :] holds
    G_sub[a][b] (upper a<=b used). Both are destroyed; xbuf[:, a, :] gets x.
    """
    A = lambda a, b: scal2[:, 5 + 5 * a + b, :]
    Arow = lambda a, b0, b1: scal2[:, 5 + 5 * a + b0:5 + 5 * a + b1, :]
    rh = lambda a: scal2[:, a, :]
    recs = [tmp_pool.tile([128, 32], f32, tag="rec%d" % i, name="rec%d" % i)
            for i in range(n)]
    if n == 1:
        nc.vector.reciprocal(recs[0][:], A(0, 0))
        nc.vector.tensor_tensor(xbuf[:, 0, :], rh(0), recs[0][:], op=Alu.mult)
        return
    f = tmp_pool.tile([128, 1, 32], f32, tag="f")
    tmp = tmp_pool.tile([128, K, 32], f32, tag="tmp")
    for i in range(n):
        nc.vector.reciprocal(recs[i][:], A(i, i))
        for j in range(i + 1, n):
            nc.vector.tensor_tensor(f[:, 0, :], A(i, j), recs[i][:], op=Alu.mult)
            m = n - j
            nc.vector.tensor_tensor(tmp[:, :m, :], Arow(i, j, n),
                                    f[:].to_broadcast([128, m, 32]), op=Alu.mult)
            nc.vector.tensor_tensor(Arow(j, j, n), Arow(j, j, n), tmp[:, :m, :],
                                    op=Alu.subtract)
            nc.vector.tensor_tensor(tmp[:, 0, :], rh(i), f[:, 0, :], op=Alu.mult)
            nc.vector.tensor_tensor(rh(j), rh(j), tmp[:, 0, :], op=Alu.subtract)
    for i in range(n - 1, -1, -1):
        m = n - 1 - i
        if m > 0:
            nc.vector.tensor_tensor(tmp[:, :m, :], Arow(i, i + 1, n),
                                    xbuf[:, i + 1:n, :], op=Alu.mult)
            v = bass.AP(tmp[:].tensor, tmp[:].offset,
                        [tmp[:].ap[0], [1, 32], [32, m]])
            nc.vector.tensor_reduce(f[:, 0, :], v, axis=Ax.X, op=Alu.add)
            nc.vector.tensor_tensor(f[:, 0, :], rh(i), f[:, 0, :], op=Alu.subtract)
            nc.vector.tensor_tensor(xbuf[:, i, :], f[:, 0, :], recs[i][:], op=Alu.mult)
        else:
            nc.vector.tensor_tensor(xbuf[:, i, :], rh(i), recs[i][:], op=Alu.mult)


def build_core_program():
    nc = bacc_mod.Bacc(None)
    z_d = nc.dram_tensor("z", [C, HW], f32, kind="ExternalInput")
    d_d = nc.dram_tensor("d", [C, N], f32, kind="ExternalInput")
    dt_d = nc.dram_tensor("dt", [N, C], f32, kind="ExternalInput")
    rec_d = nc.dram_tensor("rec", [C, HW], f32, kind="ExternalOutput")
    jout_d = nc.dram_tensor("jout", [128, K, T], u32, kind="ExternalOutput")
    xout_d = nc.dram_tensor("xout", [128, K, T], f32, kind="ExternalOutput")

    with tile.TileContext(nc) as tc:
        with (
            tc.tile_pool(name="const", bufs=1) as cp,
            tc.tile_pool(name="sb", bufs=1) as sb,
            tc.tile_pool(name="rot", bufs=3) as rot,
            tc.tile_pool(name="hps", bufs=4, space="PSUM") as hps,
            tc.tile_pool(name="tps", bufs=2, space="PSUM") as tps,
        ):
            z_sb = cp.tile([C, HW], f32)
            d_sb = cp.tile([C, N], f32)
            ident = cp.tile([128, 128], f32)
            nc.sync.dma_start(z_sb[:], z_d[:])
            nc.sync.dma_start(d_sb[:], d_d[:])
            make_identity(nc, ident[:])

            # ubuf segments: 0 = p_t (patch), 1+a = u_a (selected atoms)
            ubuf = sb.tile([128, T, 6, C], f32)
            jbuf = sb.tile([128, K, T, 8], u32)
            # rows 0..4: rhs p.u_b; rows 5+5a+b: G_sub[a][b] (upper used)
            scal2 = sb.tile([128, 30, 32], f32)
            scw = sb.tile([128, 30, 32], f32)
            xbuf = sb.tile([128, K, 32], f32)
            rT_sb = sb.tile([C, T, 128], f32)
            nc.gpsimd.memset(scal2[:], 0.0)

            for t in range(T):
                pt_ps = tps.tile([128, C], f32, tag="pt")
                nc.tensor.transpose(pt_ps[:], z_sb[:, t * 128:(t + 1) * 128],
                                    ident[0:C, 0:C])
                nc.scalar.copy(ubuf[:, t, 0, :], pt_ps[:])

            TG = 4  # patch-tiles batched per DVE/DMA instruction
            ub = ubuf[:]
            for k in range(K):
                for t0 in range(0, T, TG):
                    for t in range(t0, t0 + TG):
                        h_ps = hps.tile([128, N], f32, tag="h")
                        lhsT = (z_sb[:, t * 128:(t + 1) * 128] if k == 0
                                else rT_sb[:, t, :])
                        nc.tensor.matmul(h_ps[:], lhsT, d_sb[:],
                                         start=True, stop=True)
                        s = rot.tile([128, N], f32, tag="s", name="s", bufs=4)
                        nc.scalar.activation(s[:], h_ps[:], Act.Abs)
                        m8 = rot.tile([128, 8], f32, tag="m8", name="m8", bufs=4)
                        nc.vector.max(m8[:], s[:])
                        nc.vector.max_index(jbuf[:, k, t, :], m8[:], s[:])
                    for t in range(t0, t0 + TG):
                        nc.gpsimd.indirect_dma_start(
                            out=ubuf[:, t, k + 1, :], out_offset=None,
                            in_=dt_d[:],
                            in_offset=bass.IndirectOffsetOnAxis(
                                ap=jbuf[:, k, t, 0:1], axis=0))
                    # dots of u_k against [p, u_0..u_k] for 4 tiles:
                    # one mul + one segmented reduce, strided out so row k
                    # gets the rhs and rows 5+5a+k get G_sub[a][k].
                    prod = rot.tile([128, TG, 6, C], f32, tag="prod", name="prod")
                    nc.vector.tensor_tensor(
                        prod[:, :, 0:k + 2, :], ub[:, t0:t0 + TG, 0:k + 2, :],
                        bass.AP(ub.tensor, ub.offset + (t0 * 6 + k + 1) * C,
                                [ub.ap[0], [6 * C, TG], [0, k + 2], [1, C]]),
                        op=Alu.mult)
                    out_ap = bass.AP(scal2[:].tensor,
                                     scal2[:].offset + (k * 32 + t0),
                                     [scal2[:].ap[0], [1, TG], [5 * 32, k + 2]])
                    nc.vector.tensor_reduce(out_ap, prod[:, :, 0:k + 2, :],
                                            axis=Ax.X, op=Alu.add)
                # batched solve across all 32 tiles
                dpos = 5 + 6 * k
                nc.vector.tensor_scalar(scal2[:, dpos, :], scal2[:, dpos, :],
                                        EPS, None, op0=Alu.add)
                nc.vector.tensor_copy(scw[:], scal2[:])
                _emit_solve(nc, scw, xbuf, rot, k + 1)
                for t0 in range(0, T, TG):
                    # c = sum_a x_a u_a for 4 tiles: mul + segment reduce
                    prod2 = rot.tile([128, TG, K, C], f32, tag="prod2",
                                     name="prod2")
                    xb = xbuf[:]
                    nc.vector.tensor_tensor(
                        prod2[:, :, 0:k + 1, :], ub[:, t0:t0 + TG, 1:k + 2, :],
                        bass.AP(xb.tensor, xb.offset + t0,
                                [xb.ap[0], [1, TG], [32, k + 1], [0, C]]),
                        op=Alu.mult)
                    ct4 = rot.tile([128, TG, C], f32, tag="ct4", name="ct4")
                    p2 = prod2[:]
                    v = bass.AP(p2.tensor, p2.offset,
                                [p2.ap[0], [K * C, TG], [1, C], [C, k + 1]])
                    nc.vector.tensor_reduce(ct4[:], v, axis=Ax.X, op=Alu.add)
                    if k < K - 1:
                        rt4 = rot.tile([128, TG, C], f32, tag="rt4", name="rt4")
                        nc.vector.tensor_tensor(rt4[:], ub[:, t0:t0 + TG, 0, :],
                                                ct4[:], op=Alu.subtract)
                        for i in range(TG):
                            rt_ps = tps.tile([C, 128], f32, tag="rt")
                            nc.tensor.transpose(rt_ps[:], rt4[:, i, :], ident[:])
                            nc.scalar.copy(rT_sb[:, t0 + i, :], rt_ps[:])
                    else:
                        for i in range(TG):
                            rt_ps = tps.tile([C, 128], f32, tag="rt")
                            nc.tensor.transpose(rt_ps[:], ct4[:, i, :], ident[:])
                            crec = rot.tile([C, 128], f32, tag="crec",
                                            name="crec")
                            nc.scalar.copy(crec[:], rt_ps[:])
                            t = t0 + i
                            nc.sync.dma_start(rec_d[:, t * 128:(t + 1) * 128],
                                              crec[:])

            nc.sync.dma_start(jout_d[:], jbuf[:, :, :, 0])
            nc.sync.dma_start(xout_d[:], xbuf[:])
    nc.finalize()
    return nc


_CACHED_NC = None


def kernel(z_e: np.ndarray, dictionary: np.ndarray):
    global _CACHED_NC
    z_e = np.ascontiguousarray(np.asarray(z_e, dtype=np.float32))
    D = np.ascontiguousarray(np.asarray(dictionary, dtype=np.float32))
    DT = np.ascontiguousarray(D.T)

    if _CACHED_NC is None:
        _CACHED_NC = build_core_program()
    nc = _CACHED_NC

    in_maps = []
    for b in range(B):
        zb = np.ascontiguousarray(z_e[b].reshape(C, HW))
        in_maps.append({"z": zb, "d": D, "dt": DT})
    res = run_bass_kernel_spmd(nc, in_maps, core_ids=list(range(B))).results

    z_dl = np.empty((B, C, 64, 64), dtype=np.float32)
    coeffsT = np.zeros((N, B * HW), dtype=np.float32)
    pidx = np.arange(HW)
    for b in range(B):
        rec = res[b]["rec"]                       # [C, HW]
        z_dl[b] = rec.reshape(C, 64, 64)
        jout = res[b]["jout"].astype(np.int64)    # [128, K, T]
        xout = res[b]["xout"]                     # [128, K, T]
        # patch p = t*128 + q  (q = partition)
        j_flat = jout.transpose(2, 0, 1).reshape(HW, K)   # [p, k]
        x_flat = xout.transpose(2, 0, 1).reshape(HW, K)
        for k in range(K):
            coeffsT[j_flat[:, k], b * HW + pidx] = x_flat[:, k]

    diff = (z_dl - z_e).astype(np.float32)
    sq = diff * diff
    loss = np.float32((1.0 + COMMIT) * np.mean(sq, dtype=np.float32))
    return z_dl, loss, coeffsT
